# revision 10
# baseline (speedup 1.0000x reference)
"""Trainium2 Bass kernel for GQA attention block (nn_Attention_6219112644965).

Reference computation (per batch b):
  q = rope(rmsnorm(x @ Wq, q_gamma), cos, sin)   # 16 heads x 128
  k = rope(rmsnorm(x @ Wk, k_gamma), cos, sin)   # 8 kv heads x 128
  v = x @ Wv
  o = softmax(q k^T / sqrt(128)) v               # GQA: q head h uses kv head h//2
  y = o @ Wo
Sharding: 8 cores = 4 batches x 2 head-groups; host sums the two partial
y's per batch. Head-dim-major on-chip layout (no transposes); rmsnorm scale
via exp(-0.5 ln(ssq+eps)) with one ACT table set; rope pair-shuffle as a
64-partition half swap with gamma/sign folded into host tables.

v2 scheduling: the whole kernel is emitted as one PE-dense stream.
 - ~3.5us of warmup matmuls on zeros ramp the HAM clock gate while inputs DMA.
 - xt is DMA'd in 512-column chunks in consumption order.
 - Phase B: k heads, v, q0, q1 back-to-back (PE-dense, ACT light).
 - Phase C: per head h: attention chunk0 with the projection of head h+2
   interleaved into every tk step (fills the exp-paced PE slack).
 - Phase D: per head h: attention chunk1 with out-projection row-tile h
   interleaved per tk step.
 - Phase E: remaining 8 out-projection row tiles on a 4-deep PSUM ring.
 - All [1,512] partition-reduce matmuls (rmsnorm ssq, softmax denominator)
   are packed 4x/2x into single PE slots via tile_position column packing.
 - One ACT table set for the entire kernel (Exp/Ln/Square/Copy all pinned to
   natural_log_exp_and_others), loaded during the DMA lead-in.
"""
import sys

sys.path.insert(0, "/opt/trn_rl_repo")

from contextlib import ExitStack

import ml_dtypes
import numpy as np

import bass_rust
import concourse.bass as bass
import concourse.mybir as mybir
import concourse.tile as tile
from concourse import bacc, hw_specs
from concourse.bass_utils import run_bass_kernel_spmd

F32 = mybir.dt.float32
BF16 = mybir.dt.bfloat16
AF = mybir.ActivationFunctionType

T = 2048          # sequence length
D = 2048          # model dim
HD = 128          # head dim
NQH = 8           # q heads per core
NKV = 4           # kv heads per core
ND = D // 128     # 16 d-tiles
NTT = T // 128    # 16 t-tiles
TQC = 1024        # attention tq chunk
EPS = 1e-6

_CACHE = {}
LAST_RESULTS = None

_PINNED = {AF.Exp, AF.Ln, AF.Square, AF.Copy, AF.Identity}


class _Bacc(bacc.Bacc):
    """Bacc with every ACT func the kernel uses pinned to one table set.

    Hiding Exp/Ln/Square/Copy/Identity from all sets except
    `natural_log_exp_and_others` (which contains them all) makes every
    activation resolve to that set: a single ACT_TABLE_LOAD for the whole
    kernel, issued before the first activation (inside the DMA lead-in).
    """

    def insert_act_table_loads(self):
        has_activation = any(
            isinstance(i, mybir.InstActivation)
            for b in self.main_func.blocks
            for i in b.instructions
        )
        if not has_activation:
            return
        tables = []
        for name, funcs in hw_specs.get_activation_tables(self.m.arch).items():
            if name != "natural_log_exp_and_others":
                funcs = funcs - _PINNED
            tables.append((name, funcs))
        bass_rust.insert_act_table_loads(self, tables)


def build_module():
    """Build the per-core Bass program (identical on all 8 cores)."""
    nc = _Bacc("TRN2", target_bir_lowering=False, debug=False)

    # ---- DRAM I/O (host-packed so every DMA is contiguous) ----
    xt_d = nc.dram_tensor("xt", [128, ND, T], BF16, kind="ExternalInput")
    wq_d = nc.dram_tensor("wq", [NQH, 128, ND, HD], BF16, kind="ExternalInput")
    wk_d = nc.dram_tensor("wk", [NKV, 128, ND, HD], BF16, kind="ExternalInput")
    wv_d = nc.dram_tensor("wv", [128, ND, NKV * HD], BF16, kind="ExternalInput")
    wo_d = nc.dram_tensor("wo", [128, NQH, D], BF16, kind="ExternalInput")
    cosq_d = nc.dram_tensor("cosq", [128, T], BF16, kind="ExternalInput")
    sinq_d = nc.dram_tensor("sinq", [128, T], BF16, kind="ExternalInput")
    cosk_d = nc.dram_tensor("cosk", [128, T], BF16, kind="ExternalInput")
    sink_d = nc.dram_tensor("sink", [128, T], BF16, kind="ExternalInput")
    y_d = nc.dram_tensor("y", [T, D], F32, kind="ExternalOutput")

    with tile.TileContext(nc) as tc, ExitStack() as top:
        persist = top.enter_context(tc.tile_pool(name="persist", bufs=1))
        qT = persist.tile([128, NQH, T], BF16, tag="qT")
        kT = persist.tile([128, NKV, T], BF16, tag="kT")
        v_sb = persist.tile([128, NTT, NKV * HD], BF16, tag="v")
        ones_col = persist.tile([128, 1], BF16, tag="ones")
        nc.vector.memset(ones_col, 1.0)
        ones32 = persist.tile([128, 32], BF16, tag="ones32")
        nc.vector.memset(ones32, 1.0)
        zero128 = persist.tile([128, 1], F32, tag="zero128")
        nc.vector.memset(zero128, 0.0)
        epsq = persist.tile([128, 1], F32, tag="epsq")
        nc.vector.memset(epsq, float(HD * EPS))
        epsk = persist.tile([128, 1], F32, tag="epsk")
        nc.vector.memset(epsk, float(EPS))

        # ---------------- warmup: ramp the HAM clock gate ----------------
        with ExitStack() as warm_scope:
            wpool = warm_scope.enter_context(tc.tile_pool(name="warm", bufs=1))
            wps = warm_scope.enter_context(
                tc.tile_pool(name="warmps", bufs=1, space="PSUM"))
            wz = wpool.tile([128, 512], BF16, tag="wz")
            nc.vector.memset(wz, 0.0)
            warm_ps = wps.tile([128, 512], F32, tag="wps")
            for i in range(16):
                nc.tensor.matmul(warm_ps, wz[:, 0:128], wz,
                                 start=(i == 0), stop=(i == 15))
            wsink = wpool.tile([1, 1], F32, tag="wsink")
            nc.vector.tensor_copy(wsink, warm_ps[0:1, 0:1])

        es_B0 = ExitStack()     # cosk/sink/wv: closes after phase B
        es_B = ExitStack()      # xt / tables / proj work: closes mid-C
        es_D = ExitStack()      # wo / oT1 / ysb / attnD / ps_y: closes at end
        try:
            pB = es_B.enter_context(tc.tile_pool(name="pB", bufs=1))
            xt = pB.tile([128, ND, T], BF16, tag="xt")
            cosq = pB.tile([128, T], BF16, tag="cosq")
            sinq = pB.tile([128, T], BF16, tag="sinq")
            wslab_p = es_B.enter_context(tc.tile_pool(name="wslab", bufs=2))
            rawp = es_B.enter_context(tc.tile_pool(name="rawp", bufs=4))
            sqp = es_B.enter_context(tc.tile_pool(name="sqp", bufs=4))
            lrp = es_B.enter_context(tc.tile_pool(name="lrp", bufs=1))
            mwork = es_B.enter_context(tc.tile_pool(name="mwork", bufs=2))
            bcp = es_B.enter_context(tc.tile_pool(name="bcp", bufs=1))
            ps_mm = es_B.enter_context(tc.tile_pool(name="ps_mm", bufs=2,
                                                    space="PSUM"))
            pB0 = es_B0.enter_context(tc.tile_pool(name="pB0", bufs=1))
            cosk = pB0.tile([128, T], BF16, tag="cosk")
            sink = pB0.tile([128, T], BF16, tag="sink")
            wv_sb = pB0.tile([128, ND, NKV * HD], BF16, tag="wv")

            # ---- input DMA in consumption order ----
            wsl0 = wslab_p.tile([128, ND, HD], BF16, tag="wsl")
            for q4 in range(4):
                nc.sync.dma_start(out=wsl0[:, 4 * q4:4 * (q4 + 1), :],
                                  in_=wk_d[0, :, 4 * q4:4 * (q4 + 1), :])
            nc.sync.dma_start(out=cosk[:], in_=cosk_d[:])
            nc.sync.dma_start(out=sink[:], in_=sink_d[:])
            for d in range(ND):     # chunks 0,1 first (first pair-group)
                nc.sync.dma_start(out=xt[:, d, 0:512], in_=xt_d[:, d, 0:512])
                nc.sync.dma_start(out=xt[:, d, 512:1024],
                                  in_=xt_d[:, d, 512:1024])
            for d in range(ND):
                nc.sync.dma_start(out=xt[:, d, 1024:1536],
                                  in_=xt_d[:, d, 1024:1536])
                nc.sync.dma_start(out=xt[:, d, 1536:2048],
                                  in_=xt_d[:, d, 1536:2048])
            nc.sync.dma_start(out=cosq[:], in_=cosq_d[:])
            nc.sync.dma_start(out=sinq[:], in_=sinq_d[:])
            nc.sync.dma_start(out=wv_sb[:], in_=wv_d[:])

            def load_wslab(h, w_dram):
                wsl = wslab_p.tile([128, ND, HD], BF16, tag="wsl")
                nc.sync.dma_start(out=wsl[:], in_=w_dram[h])
                return wsl

            def qk_proj_steps(h, out_T, cos_t, sin_t, is_q, wsl):
                """Yield PE-quantum callables for one head's projection.

                Each yielded callable emits ~2-4 matmuls (plus the non-PE
                epilogue work attached to the last quantum of each stage).
                """
                pair_state = {}

                def mm_quantum(pair, dd):
                    def emit():
                        if dd == 0:
                            pair_state["a"] = ps_mm.tile([128, 512], F32, tag="mm", name="mm_a")
                            pair_state["b"] = ps_mm.tile([128, 512], F32, tag="mm", name="mm_b")
                        ps_a, ps_b = pair_state["a"], pair_state["b"]
                        c0 = slice(pair * 1024, pair * 1024 + 512)
                        c1 = slice(pair * 1024 + 512, (pair + 1) * 1024)
                        for d in (dd, dd + 1):
                            nc.tensor.matmul(ps_a, wsl[:, d, :], xt[:, d, c0],
                                             start=(d == 0), stop=(d == ND - 1))
                            nc.tensor.matmul(ps_b, wsl[:, d, :], xt[:, d, c1],
                                             start=(d == 0), stop=(d == ND - 1))
                        if dd == ND - 2:  # pair done: raw copies + squares
                            for j, ps in enumerate((ps_a, ps_b)):
                                c = 2 * pair + j
                                raw = rawp.tile([128, 512], BF16, tag="raw", name="raw")
                                nc.vector.tensor_copy(raw, ps)
                                pair_state[f"raw{c}"] = raw
                                sq = sqp.tile([128, 512], BF16, tag="sq", name="sq")
                                nc.scalar.activation(out=sq, in_=ps,
                                                     func=AF.Square,
                                                     bias=zero128[:, :])
                                pair_state[f"sq{c}"] = sq
                    return emit

                for pair in range(2):
                    for dd in range(0, ND, 2):
                        yield mm_quantum(pair, dd)

                def epilogue():
                    # packed ssq: 4 col-tiled [1,512] matmuls in one PE slot
                    ssq_ps = ps_mm.tile([128, 512], F32, tag="mm")
                    for c in range(4):
                        # ones32 stationary: 32 identical rows per chunk, so
                        # every partition of the bank is written (clean reads)
                        nc.tensor.matmul(ssq_ps[32 * c:32 * (c + 1), :],
                                         ones32, pair_state[f"sq{c}"],
                                         start=True, stop=True,
                                         tile_position=(0, 32 * c))
                    lnr = lrp.tile([128, 512], F32, tag="lnr")
                    if is_q:
                        nc.scalar.activation(out=lnr, in_=ssq_ps, func=AF.Ln,
                                             scale=1.0, bias=epsq[:, :])
                    else:
                        nc.scalar.activation(out=lnr, in_=ssq_ps, func=AF.Ln,
                                             scale=1.0 / HD, bias=epsk[:, :])
                    rec = lrp.tile([128, 512], F32, tag="rec")
                    nc.scalar.activation(out=rec, in_=lnr, func=AF.Exp,
                                         scale=-0.5, bias=zero128[:, :])
                    for c in range(4):
                        cs = slice(c * 512, (c + 1) * 512)
                        bc = bcp.tile([128, 512], F32, tag="bc")
                        nc.gpsimd.partition_broadcast(bc, rec[32 * c:32 * c + 1, :])
                        raw = pair_state[f"raw{c}"]
                        m1 = mwork.tile([128, 512], BF16, tag="m1")
                        nc.vector.tensor_mul(m1, raw, cos_t[:, cs])
                        swp = mwork.tile([128, 512], BF16, tag="swp")
                        nc.vector.tensor_copy(swp[0:64, :], raw[64:128, :])
                        nc.vector.tensor_copy(swp[64:128, :], raw[0:64, :])
                        m2 = mwork.tile([128, 512], BF16, tag="m2")
                        nc.vector.tensor_mul(m2, swp, sin_t[:, cs])
                        m3 = mwork.tile([128, 512], BF16, tag="swp", name="m3")
                        nc.vector.tensor_add(m3, m1, m2)
                        nc.vector.tensor_mul(out_T[:, h, cs], m3, bc)
                yield epilogue

            def run_all(gen):
                for fn in gen:
                    fn()

            # ---------------- phase B: k heads, v, q0, q1 ----------------
            wsl_next = {}
            for kv in range(NKV):
                wsl = wsl0 if kv == 0 else wsl_next.pop(kv)
                if kv + 1 < NKV:
                    wsl_next[kv + 1] = load_wslab(kv + 1, wk_d)
                else:
                    wsl_next[0] = load_wslab(0, wq_d)  # q head 0 slab
                run_all(qk_proj_steps(kv, kT, cosk, sink, False, wsl))

            for tt in range(NTT):
                v_ps = ps_mm.tile([128, 512], F32, tag="mm")
                ts_ = slice(tt * 128, (tt + 1) * 128)
                for d in range(ND):
                    nc.tensor.matmul(v_ps, xt[:, d, ts_], wv_sb[:, d, :],
                                     start=(d == 0), stop=(d == ND - 1))
                nc.scalar.copy(v_sb[:, tt, :], v_ps)

            for h in (0, 1):
                wsl = wsl_next.pop(h)
                wsl_next[h + 1] = load_wslab(h + 1, wq_d)
                run_all(qk_proj_steps(h, qT, cosq, sinq, True, wsl))

            es_B0.close()  # free cosk/sink/wv

            # ---------------- attention ----------------
            oT0 = persist.tile([128, NQH, TQC], BF16, tag="oT0")

            def attn_pools(es, sfx, p_bufs):
                return dict(
                    ps_s=es.enter_context(
                        tc.tile_pool(name="ps_s" + sfx, bufs=2, space="PSUM")),
                    ps_o=es.enter_context(
                        tc.tile_pool(name="ps_o" + sfx, bufs=1, space="PSUM")),
                    pwork=es.enter_context(
                        tc.tile_pool(name="ppool" + sfx, bufs=p_bufs)),
                    awork=es.enter_context(
                        tc.tile_pool(name="awork" + sfx, bufs=2)),
                    nwork=es.enter_context(
                        tc.tile_pool(name="nwork" + sfx, bufs=1)),
                )

            def attn_unit(pools, h, c, oT_dst, filler):
                """One head x one tq chunk; filler() emits PE work into the
                exp-paced slack of each tk step."""
                ps_s, ps_o = pools["ps_s"], pools["ps_o"]
                pwork, awork, nwork = (pools["pwork"], pools["awork"],
                                       pools["nwork"])
                kv = h // 2
                cs0 = slice(c * TQC, c * TQC + 512)
                cs1 = slice(c * TQC + 512, (c + 1) * TQC)
                o_ps = ps_o.tile([128, TQC], F32, tag="o")
                colsum = awork.tile([128, TQC], BF16, tag="colsum")
                for tk in range(NTT):
                    ks = slice(tk * 128, (tk + 1) * 128)
                    s_ps = ps_s.tile([128, TQC], F32, tag="s")
                    nc.tensor.matmul(s_ps[:, 0:512], kT[:, kv, ks],
                                     qT[:, h, cs0], start=True, stop=True)
                    nc.tensor.matmul(s_ps[:, 512:TQC], kT[:, kv, ks],
                                     qT[:, h, cs1], start=True, stop=True)
                    p_bf = pwork.tile([128, TQC], BF16, tag="p")
                    nc.scalar.activation(out=p_bf, in_=s_ps, func=AF.Exp,
                                         bias=zero128[:, :])
                    filler(tk)
                    vt = v_sb[:, tk, kv * HD:(kv + 1) * HD]
                    nc.tensor.matmul(o_ps[:, 0:512], vt, p_bf[:, 0:512],
                                     start=(tk == 0), stop=(tk == NTT - 1))
                    nc.tensor.matmul(o_ps[:, 512:TQC], vt, p_bf[:, 512:TQC],
                                     start=(tk == 0), stop=(tk == NTT - 1))
                    if tk == 0:
                        nc.vector.tensor_copy(colsum, p_bf)
                    else:
                        nc.vector.tensor_add(colsum, colsum, p_bf)
                # free the o_ps PSUM slot immediately (unnormalized copy)
                oTun = awork.tile([128, TQC], BF16, tag="oTun")
                nc.vector.tensor_copy(oTun, o_ps)
                # packed denominator: two [1,512] matmuls in one PE slot
                den = ps_s.tile([128, TQC], F32, tag="s")
                nc.tensor.matmul(den[0:1, 0:512], ones_col, colsum[:, 0:512],
                                 start=True, stop=True, tile_position=(0, 0))
                nc.tensor.matmul(den[32:33, 0:512], ones_col, colsum[:, 512:TQC],
                                 start=True, stop=True, tile_position=(0, 32))
                recr = nwork.tile([1, TQC], F32, tag="recr")
                nc.vector.reciprocal_approx_fast(out=recr[:, 0:512],
                                                 in_=den[0:1, 0:512])
                nc.vector.reciprocal_approx_fast(out=recr[:, 512:TQC],
                                                 in_=den[32:33, 0:512])
                bc = nwork.tile([128, TQC], F32, tag="abc")
                nc.gpsimd.partition_broadcast(bc, recr)
                nc.vector.tensor_mul(oT_dst[:, h, :], oTun, bc)

            # ---- phase C: chunk0 with q2..q7 projection as filler ----
            def make_proj_filler(h):
                if h + 2 <= NQH - 1:
                    wsl = wsl_next.pop(h + 2)
                    if h + 3 <= NQH - 1:
                        wsl_next[h + 3] = load_wslab(h + 3, wq_d)
                    gen = qk_proj_steps(h + 2, qT, cosq, sinq, True, wsl)
                    quanta = list(gen)  # 17 quanta (16 mm + epilogue)
                else:
                    quanta = []

                def filler(tk):
                    # 17 quanta over 16 tk slots: slot 0 takes two
                    take = 2 if (tk == 0 and len(quanta) > 16) else 1
                    for _ in range(take):
                        if quanta:
                            quanta.pop(0)()
                return filler

            es_attnC = ExitStack()
            poolsC = attn_pools(es_attnC, "C", 2)
            for h in range(NQH):
                attn_unit(poolsC, h, 0, oT0, make_proj_filler(h))
            es_attnC.close()
            es_B.close()  # free xt/tables/proj pools + ps_mm banks

            # ---- phase D: chunk1 with out-projection tt=h as filler ----
            pD = es_D.enter_context(tc.tile_pool(name="pD", bufs=1))
            oT1 = pD.tile([128, NQH, TQC], BF16, tag="oT1")
            wo_sb = pD.tile([128, NQH, D], BF16, tag="wo")
            nc.sync.dma_start(out=wo_sb[:], in_=wo_d[:])
            ysb_p = es_D.enter_context(tc.tile_pool(name="ysb", bufs=4))
            poolsD = attn_pools(es_D, "D", 3)
            ps_y = es_D.enter_context(
                tc.tile_pool(name="ps_y", bufs=2, space="PSUM"))

            def oT_at(tt):
                """(oT tile, column slice) holding out-row-tile tt."""
                if tt < 8:
                    return oT0, slice(tt * 128, (tt + 1) * 128)
                return oT1, slice((tt - 8) * 128, (tt - 7) * 128)

            def make_outproj_filler(tt):
                """32 matmuls of out-row-tile tt: 2 per tk slot."""
                state = {}

                def filler(tk):
                    np_ = tk // 8
                    if tk % 8 == 0:
                        state["y0"] = ps_y.tile([128, 512], F32, tag="y", name="y0")
                        state["y1"] = ps_y.tile([128, 512], F32, tag="y", name="y1")
                        if np_ == 0:
                            state["ysb"] = ysb_p.tile([128, D], F32, tag="ysb", name="ysbD")
                    oT_t, ts_ = oT_at(tt)
                    h = tk % 8
                    ns0 = slice(np_ * 1024, np_ * 1024 + 512)
                    ns1 = slice(np_ * 1024 + 512, (np_ + 1) * 1024)
                    nc.tensor.matmul(state["y0"], oT_t[:, h, ts_],
                                     wo_sb[:, h, ns0],
                                     start=(h == 0), stop=(h == NQH - 1))
                    nc.tensor.matmul(state["y1"], oT_t[:, h, ts_],
                                     wo_sb[:, h, ns1],
                                     start=(h == 0), stop=(h == NQH - 1))
                    if h == NQH - 1:
                        y_sb = state["ysb"]
                        nc.vector.tensor_copy(y_sb[:, ns0], state["y0"])
                        nc.vector.tensor_copy(y_sb[:, ns1], state["y1"])
                        if np_ == 1:
                            rs = slice(tt * 128, (tt + 1) * 128)
                            nc.sync.dma_start(out=y_d[rs, 0:1024],
                                              in_=y_sb[:, 0:1024])
                            nc.sync.dma_start(out=y_d[rs, 1024:D],
                                              in_=y_sb[:, 1024:D])
                return filler

            for h in range(NQH):
                attn_unit(poolsD, h, 1, oT1, make_outproj_filler(h))

            # ---- phase E: out-projection tail ----
            # reuse poolsD's s-tiles ([128,1024] = 2 banks) as psum pairs:
            # together with ps_y this gives a 6-deep effective ring.
            ps_sD = poolsD["ps_s"]
            for tt in range(8, NTT):
                oT_t, ts_ = oT_at(tt)
                y_sb = ysb_p.tile([128, D], F32, tag="ysb")
                for np_ in range(2):
                    ns0 = slice(np_ * 1024, np_ * 1024 + 512)
                    ns1 = slice(np_ * 1024 + 512, (np_ + 1) * 1024)
                    s_pair = ps_sD.tile([128, TQC], F32, tag="s", name="ypair")
                    y_ps0 = s_pair[:, 0:512]
                    y_ps1 = s_pair[:, 512:TQC]
                    for h in range(NQH):
                        nc.tensor.matmul(y_ps0, oT_t[:, h, ts_],
                                         wo_sb[:, h, ns0],
                                         start=(h == 0), stop=(h == NQH - 1))
                        nc.tensor.matmul(y_ps1, oT_t[:, h, ts_],
                                         wo_sb[:, h, ns1],
                                         start=(h == 0), stop=(h == NQH - 1))
                    nc.scalar.copy(y_sb[:, ns0], y_ps0)
                    nc.vector.tensor_copy(y_sb[:, ns1], y_ps1)
                rs = slice(tt * 128, (tt + 1) * 128)
                nc.sync.dma_start(out=y_d[rs, 0:1024], in_=y_sb[:, 0:1024])
                nc.sync.dma_start(out=y_d[rs, 1024:D], in_=y_sb[:, 1024:D])
        finally:
            es_D.close()

    nc.compile()
    return nc


def _get_module():
    if "nc" not in _CACHE:
        _CACHE["nc"] = build_module()
    return _CACHE["nc"]


def _pack_inputs(x, cos, sin, Wq, Wk, Wv, Wo, q_gamma, k_gamma):
    """Host-side prep: per-core input dicts with bf16 packed layouts."""
    bf16 = ml_dtypes.bfloat16
    perm = np.concatenate([np.arange(0, HD, 2), np.arange(1, HD, 2)])  # [128]
    partner = np.concatenate([perm[64:], perm[:64]])
    sign = np.concatenate([-np.ones(64), np.ones(64)]).astype(np.float32)

    cosT = np.ascontiguousarray(cos.T)  # [128, T]
    sinT = np.ascontiguousarray(sin.T)

    def tables(gamma):
        c = (cosT[perm] * gamma[perm][:, None]).astype(bf16)
        s = (sinT[perm] * sign[:, None] * gamma[partner][:, None]).astype(bf16)
        return np.ascontiguousarray(c), np.ascontiguousarray(s)

    cosq, sinq = tables(q_gamma.astype(np.float32))
    cosk, sink = tables(k_gamma.astype(np.float32))

    per_hg = []
    for hg in range(2):
        qh = slice(hg * NQH * HD, (hg + 1) * NQH * HD)
        kh = slice(hg * NKV * HD, (hg + 1) * NKV * HD)
        wq = Wq[:, qh].reshape(ND, 128, NQH, HD)[..., perm]
        wq = np.ascontiguousarray(wq.transpose(2, 1, 0, 3)).astype(bf16)
        wk = Wk[:, kh].reshape(ND, 128, NKV, HD)[..., perm]
        wk = np.ascontiguousarray(wk.transpose(2, 1, 0, 3)).astype(bf16)
        wv = Wv[:, kh].reshape(ND, 128, NKV * HD)
        wv = np.ascontiguousarray(wv.transpose(1, 0, 2)).astype(bf16)
        wo = Wo[hg * NQH * HD:(hg + 1) * NQH * HD, :].reshape(NQH, 128, D)
        wo = np.ascontiguousarray(wo.transpose(1, 0, 2)).astype(bf16)
        per_hg.append(dict(wq=wq, wk=wk, wv=wv, wo=wo))

    in_maps = []
    for b in range(4):
        xt = np.ascontiguousarray(
            x[b].T.reshape(ND, 128, T).transpose(1, 0, 2)).astype(bf16)
        for hg in range(2):
            m = dict(xt=xt, cosq=cosq, sinq=sinq, cosk=cosk, sink=sink,
                     **per_hg[hg])
            in_maps.append(m)
    return in_maps


def kernel(x, cos, sin, Wq, Wk, Wv, Wo, q_gamma, k_gamma, **run_kwargs):
    global LAST_RESULTS
    args = [np.asarray(a, dtype=np.float32)
            for a in (x, cos, sin, Wq, Wk, Wv, Wo, q_gamma, k_gamma)]
    nc = _get_module()
    in_maps = _pack_inputs(*args)
    res = run_bass_kernel_spmd(nc, in_maps, core_ids=list(range(8)), **run_kwargs)
    LAST_RESULTS = res
    y = np.empty((4, T, D), dtype=np.float32)
    for b in range(4):
        y[b] = np.asarray(res.results[2 * b]["y"]) + np.asarray(res.results[2 * b + 1]["y"])
    return y
